# revision 13
# baseline (speedup 1.0000x reference)
"""CrystalGraphALIGNN Trainium2 kernel (8 NeuronCores, SPMD) — v2.

Strategy: dst-shard edges across cores (atom v owned by core v // (N/8); edge
(i,j) owned by the core of its dst). Per core, edges are sorted by dst and
grouped into 128-atom blocks so that:
  - the dst-side expansion A_dst[dst(e)] is a block-local one-hot matmul,
  - the scatter-mean aggregation is a one-hot matmul into PSUM,
  - only the src side needs a true random gather: per-edge rows of
    A_src = node @ W_src, fetched with dma_gather (transposed, bf16) from a
    DRAM table that is refreshed once per layer via AllGather.
v2: the one-hot matrices are no longer shipped from the host (that was
~45 MB/core of input transfer per call). They are built on device from the
per-edge local dst index (a bf16/f32 scalar stream, ~0.5 MB/core):
  S^T [a,e]: broadcast the index row across partitions with a K=1 matmul,
  then vector is_equal against an iota column.  S [e,a]: vector is_equal of
  a constant iota matrix against the per-partition index column. The edge
  mask is applied to the transposed edge state (per-partition scale) instead
  of being baked into S. Crystal pooling likewise builds its one-hot over a
  padded 1024-crystal axis on device (iota row broadcast + is_equal with the
  per-atom crystal id), replacing the 12.5 MB/core pmat input.
Node states and the node MLP stay fully shard-local; crystal pooling ends in
a single AllReduce, readout replicated on every core.
"""

import numpy as np
import ml_dtypes

import concourse.bass as bass
import concourse.bacc as bacc
import concourse.mybir as mybir
import concourse.tile as tile
from concourse import library_config

F32 = mybir.dt.float32
BF16 = mybir.dt.bfloat16
I16 = mybir.dt.int16
AFT = mybir.ActivationFunctionType
ALU = mybir.AluOpType
BF = ml_dtypes.bfloat16

NCORES = 8
ED, ND, HID, RD = 64, 128, 128, 128
EDGE_THRESH = 1e-6
NCRYSP = 1024  # padded crystal axis for pooling one-hots
NGP = NCRYSP // 128

FULL_CFG = dict(N=50000, M=12, AFD=92, EFD=41, NCRYS=1000, L=4)


def _cdiv(a, b):
    return (a + b - 1) // b


def _wrap16(flat):
    """int16 flat idx -> [16, len/16] wrapped layout (device replicates x8)."""
    n = len(flat)
    assert n % 16 == 0
    return np.ascontiguousarray(flat.reshape(n // 16, 16).T.astype(np.int16))


def _prep(inputs, cfg):
    N, M, AFD, EFD, NCRYS, L = (cfg[k] for k in ("N", "M", "AFD", "EFD", "NCRYS", "L"))
    assert NCRYS <= NCRYSP
    ASH = N // NCORES
    NBLK = _cdiv(ASH, 128)
    LOS = min(25000, N)  # src index split for int16 gather indices

    af = np.asarray(inputs["atom_fea"], np.float32)
    nf = np.asarray(inputs["nbr_fea"], np.float32)
    nidx = np.asarray(inputs["nbr_fea_idx"]).astype(np.int64)
    cb = np.asarray(inputs["crystal_batch"]).astype(np.int64)

    E = N * M
    dst = np.clip(nidx.reshape(-1), 0, N - 1)
    src = np.repeat(np.arange(N, dtype=np.int64), M)
    ea = nf.reshape(E, EFD)
    mask = (np.abs(ea).sum(1) > EDGE_THRESH).astype(np.float32)

    cnt = np.bincount(dst, weights=mask, minlength=N)
    invcnt = (1.0 / np.maximum(cnt, 1.0)).astype(np.float32)
    ccnt = np.bincount(cb, minlength=NCRYS).astype(np.float32)
    invccnt = (1.0 / np.maximum(ccnt, 1.0)).astype(np.float32)

    core_of = dst // ASH
    dloc = dst - core_of * ASH
    blk_of = dloc // 128

    # per-core, per-block, lo/hi edge id lists
    lists = [[[None, None] for _ in range(NBLK)] for _ in range(NCORES)]
    order = np.lexsort((dst, blk_of + core_of * NBLK))  # group by (core, blk)
    g_sorted = (blk_of + core_of * NBLK)[order]
    bounds = np.searchsorted(g_sorted, np.arange(NCORES * NBLK + 1))
    for k in range(NCORES):
        for b in range(NBLK):
            eb = order[bounds[k * NBLK + b]:bounds[k * NBLK + b + 1]]
            lists[k][b][0] = eb[src[eb] < LOS]
            lists[k][b][1] = eb[src[eb] >= LOS]

    T_lo = np.zeros(NBLK, np.int64)
    T_hi = np.zeros(NBLK, np.int64)
    for b in range(NBLK):
        T_lo[b] = max(_cdiv(max(len(lists[k][b][0]) for k in range(NCORES)), 128), 1)
        T_hi[b] = _cdiv(max(len(lists[k][b][1]) for k in range(NCORES)), 128)
        if (T_lo[b] + T_hi[b]) % 2:
            if N > LOS:
                T_hi[b] += 1
            else:
                T_lo[b] += 1

    # geometry: edge-col space (block-major), state-col space (per half),
    # chunk list entries: (state_col, edge_col, blk_edge_col, n)
    BHALF = NBLK // 2
    ecol = np.zeros(NBLK + 1, np.int64)
    for b in range(NBLK):
        ecol[b + 1] = ecol[b] + (T_lo[b] + T_hi[b]) * 128
    EP = int(ecol[NBLK])
    scol = np.zeros(NBLK, np.int64)
    acc = [0, 0]
    blocks = []
    for b in range(NBLK):
        half = 0 if b < BHALF else 1
        scol[b] = acc[half]
        nb_e = (T_lo[b] + T_hi[b]) * 128
        acc[half] += nb_e
        tiles = (T_lo[b] + T_hi[b])
        chunks = []
        off = 0
        while tiles > 0:
            t = 4 if tiles >= 4 else tiles
            chunks.append((int(scol[b] + off), int(ecol[b] + off), off, t * 128))
            off += t * 128
            tiles -= t
        blocks.append(dict(b=b, half=half, nblk_e=nb_e, chunks=chunks,
                           n_lo=int(T_lo[b] * 128), n_hi=int(T_hi[b] * 128)))
    EPC = max(acc)
    IWL = sum(int(t) * 8 for t in T_lo)
    IWH = sum(int(t) * 8 for t in T_hi)

    meta = dict(cfg=cfg, ASH=ASH, NBLK=NBLK, LOS=LOS, EP=EP, EPC=EPC,
                BHALF=BHALF, blocks=blocks, IWL=IWL, IWH=IWH,
                out_b=float(np.asarray(inputs["out_b"]).reshape(-1)[0]))

    # shared weights
    eW1 = np.asarray(inputs["eW1"], np.float32)
    eW2 = np.asarray(inputs["eW2"], np.float32)
    nW1 = np.asarray(inputs["nW1"], np.float32)
    nW2 = np.asarray(inputs["nW2"], np.float32)

    def bfc(x):
        return np.ascontiguousarray(x, np.float32).astype(BF)

    we_dup = np.zeros((128, L * HID), np.float32)
    nw1b_dup = np.zeros((128, L * ND), np.float32)
    for l in range(L):
        we_dup[0:64, l * HID:(l + 1) * HID] = eW1[l, 0:ED]
        we_dup[64:128, l * HID:(l + 1) * HID] = eW1[l, 0:ED]
        nw1b_dup[0:64, l * ND:(l + 1) * ND] = nW1[l, ND:ND + ED]
        nw1b_dup[64:128, l * ND:(l + 1) * ND] = nW1[l, ND:ND + ED]
    ws_all = np.concatenate([eW1[l, ED:ED + ND] for l in range(L)], 1)      # [128, L*128]
    wd_all = np.concatenate([eW1[l, ED + ND:] for l in range(L)], 1)        # [128, L*128]
    ew2_all = np.concatenate([eW2[l] for l in range(L)], 1)                 # [128, L*64]
    nw1a_all = np.concatenate([nW1[l, 0:ND] for l in range(L)], 1)          # [128, L*128]
    nw2_all = np.concatenate([nW2[l] for l in range(L)], 1)                 # [128, L*128]

    eb1 = np.asarray(inputs["eb1"], np.float32).T.copy()                    # [128, L]
    eb2p = np.zeros((128, L), np.float32)
    eb2p[0:64] = np.asarray(inputs["eb2"], np.float32).T
    eb2p[64:128] = eb2p[0:64]
    nb1 = np.asarray(inputs["nb1"], np.float32).T.copy()
    nb2 = np.asarray(inputs["nb2"], np.float32).T.copy()

    i64d = np.zeros((128, 64), np.float32)
    i64d[0:64] = np.eye(64)
    i64d[64:128] = np.eye(64)

    atom_bc = np.asarray(inputs["atom_b"], np.float32).reshape(ND, 1)
    edge_bc = np.zeros((128, 1), np.float32)
    edge_bc[0:64, 0] = np.asarray(inputs["edge_b"], np.float32)
    edge_bc[64:128, 0] = edge_bc[0:64, 0]

    iotaM = np.broadcast_to(np.arange(128, dtype=np.float32), (128, 128)).copy()
    iota_col = np.arange(128, dtype=np.float32).reshape(128, 1)
    iota1024f = np.arange(NCRYSP, dtype=np.float32).reshape(1, NCRYSP)
    invccnt8 = np.ones((128, NGP), np.float32)
    for g in range(NGP):
        hi = min(128, NCRYS - g * 128)
        if hi > 0:
            invccnt8[0:hi, g] = invccnt[g * 128:g * 128 + hi]

    shared = {
        "atomW": bfc(np.asarray(inputs["atom_W"])), "edgeW": bfc(np.asarray(inputs["edge_W"])),
        "atom_bc": atom_bc, "edge_bc": edge_bc,
        "we_dup": bfc(we_dup), "nw1b_dup": bfc(nw1b_dup),
        "ws_all": bfc(ws_all), "wd_all": bfc(wd_all), "ew2_all": bfc(ew2_all),
        "nw1a_all": bfc(nw1a_all), "nw2_all": bfc(nw2_all),
        "readW": bfc(np.asarray(inputs["read_W"])), "outW": bfc(np.asarray(inputs["out_W"])),
        "eb1": eb1, "eb2p": eb2p, "nb1": nb1, "nb2": nb2,
        "readb": np.asarray(inputs["read_b"], np.float32).reshape(RD, 1),
        "i64d": bfc(i64d), "i128b": bfc(np.eye(128)), "i128f": np.eye(128, dtype=np.float32),
        "iotaM": iotaM, "iota_col": iota_col, "iota1024f": iota1024f,
        "ones1b": bfc(np.ones((1, 128))), "ones1f": np.ones((1, 128), np.float32),
        "invccnt8": invccnt8,
    }

    in_maps = []
    for k in range(NCORES):
        a0 = k * ASH
        eattrT = np.zeros((EFD, EP), np.float32)
        arow_slot = np.zeros(EP, np.float32)
        mask_slot = np.zeros(EP, np.float32)
        idxl = np.zeros(sum(int(t) * 128 for t in T_lo), np.int64)
        idxh = np.zeros(sum(int(t) * 128 for t in T_hi), np.int64)
        ol = oh = 0
        for blk in blocks:
            b = blk["b"]
            ids_lo, ids_hi = lists[k][b]
            n_lo, n_hi = blk["n_lo"], blk["n_hi"]
            eo = int(ecol[b])
            ids = np.full(n_lo + n_hi, -1, np.int64)
            ids[:len(ids_lo)] = ids_lo
            ids[n_lo:n_lo + len(ids_hi)] = ids_hi
            real = ids >= 0
            rids = ids[real]
            eattrT[:, eo:eo + n_lo + n_hi][:, real] = ea[rids].T
            epos = np.nonzero(real)[0]
            arow_slot[eo + epos] = (dloc[rids] - 128 * b).astype(np.float32)
            mask_slot[eo + epos] = mask[rids]
            gl = np.zeros(n_lo, np.int64)
            gl[:len(ids_lo)] = src[ids_lo]
            gh = np.zeros(n_hi, np.int64)
            gh[:len(ids_hi)] = src[ids_hi] - LOS
            idxl[ol:ol + n_lo] = gl
            idxh[oh:oh + n_hi] = gh
            ol += n_lo
            oh += n_hi

        inv_sb = np.ones((128, NBLK), np.float32)
        cbcol = np.full((128, NBLK), float(NCRYS), np.float32)
        for b in range(NBLK):
            na = min(128, ASH - 128 * b)
            inv_sb[0:na, b] = invcnt[a0 + 128 * b: a0 + 128 * b + na]
            cbcol[0:na, b] = cb[a0 + 128 * b: a0 + 128 * b + na]
        afT = np.ascontiguousarray(af[a0:a0 + ASH].T)

        m = {
            "eattrT": eattrT.astype(BF),
            "arow_r": arow_slot.reshape(1, EP).astype(BF),
            "arow_w": np.ascontiguousarray(arow_slot.reshape(EP // 128, 128).T),
            "mask_w": np.ascontiguousarray(mask_slot.reshape(EP // 128, 128).T),
            "idxlo": _wrap16(idxl), "invcnt": inv_sb, "cbcol": cbcol,
            "afT": afT.astype(BF),
        }
        if IWH:
            m["idxhi"] = _wrap16(idxh)
        m.update(shared)
        in_maps.append(m)
    return meta, in_maps


def _build(meta, act=AFT.Silu, noop=False, no_gather=False, no_coll=False):
    cfg = meta["cfg"]
    N, M, AFD, EFD, NCRYS, L = (cfg[k] for k in ("N", "M", "AFD", "EFD", "NCRYS", "L"))
    ASH, NBLK, LOS = meta["ASH"], meta["NBLK"], meta["LOS"]
    EP, EPC, blocks = meta["EP"], meta["EPC"], meta["blocks"]
    IWL, IWH = meta["IWL"], meta["IWH"]

    nc = bacc.Bacc("TRN2", target_bir_lowering=False, debug=False, num_devices=NCORES,
                   num_swdge_queues=4)

    def din(name, shape, dt):
        return nc.dram_tensor(name, shape, dt, kind="ExternalInput")

    eattrT = din("eattrT", [EFD, EP], BF16)
    arow_r = din("arow_r", [1, EP], BF16)
    idxlo = din("idxlo", [16, IWL], I16)
    idxhi = din("idxhi", [16, IWH], I16) if IWH else None
    afT = din("afT", [AFD, ASH], BF16)
    wts = {}
    for nm, sh, dt in [
        ("arow_w", [128, EP // 128], F32), ("mask_w", [128, EP // 128], F32),
        ("invcnt", [128, NBLK], F32), ("cbcol", [128, NBLK], F32),
        ("atomW", [AFD, ND], BF16), ("edgeW", [EFD, ED], BF16),
        ("atom_bc", [128, 1], F32), ("edge_bc", [128, 1], F32),
        ("we_dup", [128, L * HID], BF16), ("nw1b_dup", [128, L * ND], BF16),
        ("ws_all", [ND, L * HID], BF16), ("wd_all", [ND, L * HID], BF16),
        ("ew2_all", [HID, L * ED], BF16), ("nw1a_all", [ND, L * HID], BF16),
        ("nw2_all", [HID, L * ND], BF16), ("readW", [ND, RD], BF16),
        ("outW", [RD, 1], BF16), ("eb1", [128, L], F32), ("eb2p", [128, L], F32),
        ("nb1", [128, L], F32), ("nb2", [128, L], F32), ("readb", [RD, 1], F32),
        ("i64d", [128, 64], BF16), ("i128b", [128, 128], BF16),
        ("i128f", [128, 128], F32),
        ("iotaM", [128, 128], F32), ("iota_col", [128, 1], F32),
        ("iota1024f", [1, NCRYSP], F32),
        ("ones1b", [1, 128], BF16), ("ones1f", [1, 128], F32),
        ("invccnt8", [128, NGP], F32),
    ]:
        wts[nm] = din(nm, sh, dt)
    y = nc.dram_tensor("y", [1, NCRYS], F32, kind="ExternalOutput")

    if noop:
        with tile.TileContext(nc) as tc:
            with tc.tile_pool(name="sbz", bufs=1) as sbz:
                yz = sbz.tile([1, NCRYS], F32, tag="yz")
                nc.gpsimd.memset(yz[:], 0.0)
                nc.sync.dma_start(y[:], yz[:])
        nc.compile()
        return nc

    with tile.TileContext(nc) as tc:
        with (
            tc.tile_pool(name="persist", bufs=1) as pp,
            tc.tile_pool(name="dram", bufs=1, space="DRAM") as dp,
        ):
            nc.gpsimd.load_library(library_config.mlp)
            w = {nm: pp.tile(t.shape, t.dtype, tag=nm, name=f"w_{nm}") for nm, t in wts.items()}
            for nm, t in wts.items():
                nc.sync.dma_start(w[nm][:], t[:])
            stateT = pp.tile([128, EPC], BF16, tag="stateT")
            nodeT = pp.tile([128, ASH], F32, tag="nodeT")
            nodeTb = pp.tile([128, ASH], BF16, tag="nodeTb")
            adst = pp.tile([128, NBLK * 128], BF16, tag="adst")
            aggF = pp.tile([64, NBLK * 128], BF16, tag="aggF")
            idxsb = pp.tile([128, IWL], I16, tag="idxsb")
            for r in range(8):
                nc.sync.dma_start(idxsb[16 * r:16 * r + 16, :], idxlo[:])
            if IWH:
                idxsbh = pp.tile([128, IWH], I16, tag="idxsbh")
                for r in range(8):
                    nc.sync.dma_start(idxsbh[16 * r:16 * r + 16, :], idxhi[:])
            asrc_in = dp.tile([ASH, ND], BF16)
            asrc_fulls = [dp.tile([N, ND], BF16, addr_space="Shared", name=f"asrc_full{i}", tag=f"asrc_full{i}")
                          for i in range(L)]
            pool_in = dp.tile([NCRYSP, ND], F32)
            pool_out = dp.tile([NCRYSP, ND], F32, addr_space="Shared")
            iotaB = pp.tile([128, NCRYSP], F32, tag="iotaB")

            def node_tables(lw, sbp, psp):
                """A_src shard -> bounce -> AllGather; A_dst blocks (layer lw)."""
                for t in range(NBLK):
                    na = min(128, ASH - 128 * t)
                    lhs = nodeTb[:, 128 * t:128 * t + na]
                    ps_s = psp.tile([128, 128], F32, tag="ps_s")
                    nc.tensor.matmul(ps_s[0:na, :], lhs, w["ws_all"][:, lw * HID:(lw + 1) * HID],
                                     start=True, stop=True)
                    asb = sbp.tile([128, 128], BF16, tag="asb")
                    nc.vector.tensor_copy(asb[0:na, :], ps_s[0:na, :])
                    nc.sync.dma_start(asrc_in[128 * t:128 * t + na, :], asb[0:na, :])
                    ps_d = psp.tile([128, 128], F32, tag="ps_d")
                    nc.tensor.matmul(ps_d[0:na, :], lhs, w["wd_all"][:, lw * HID:(lw + 1) * HID],
                                     start=True, stop=True)
                    nc.vector.tensor_copy(adst[0:na, 128 * t:128 * t + 128][:, 0:128],
                                          ps_d[0:na, :])
                if not no_coll:
                    nc.gpsimd.collective_compute(
                        "AllGather", mybir.AluOpType.bypass,
                        replica_groups=[list(range(NCORES))],
                        ins=[asrc_in[:].opt()], outs=[asrc_fulls[lw][:].opt()],
                    )

            # ---- init: projections + layer-0 tables ----
            with tc.tile_pool(name="sbi", bufs=3) as sbp, \
                 tc.tile_pool(name="psi", bufs=2, space="PSUM") as psp:
                for t in range(NBLK):
                    na = min(128, ASH - 128 * t)
                    aft = sbp.tile([AFD, 128], BF16, tag="aft")
                    nc.sync.dma_start(aft[:, 0:na], afT[:, 128 * t:128 * t + na])
                    ps_n = psp.tile([128, 128], F32, tag="ps_n")
                    nc.tensor.matmul(ps_n[:, 0:na], w["atomW"][:], aft[:, 0:na],
                                     start=True, stop=True)
                    nc.scalar.activation(nodeT[:, 128 * t:128 * t + na], ps_n[:, 0:na],
                                         AFT.Identity, bias=w["atom_bc"][:, 0:1])
                    nc.scalar.activation(nodeTb[:, 128 * t:128 * t + na], ps_n[:, 0:na],
                                         AFT.Identity, bias=w["atom_bc"][:, 0:1])
                for blk in blocks:
                    hr = slice(64, 128) if blk["half"] else slice(0, 64)
                    for (sco, eco, bco, n) in blk["chunks"]:
                        eat = sbp.tile([EFD, 512], BF16, tag="eat")
                        nc.sync.dma_start(eat[:, 0:n], eattrT[:, eco:eco + n])
                        ps_e = psp.tile([128, 512], F32, tag="ps_e")
                        nc.tensor.matmul(ps_e[hr, 0:n], w["edgeW"][:], eat[:, 0:n],
                                         start=True, stop=True)
                        nc.scalar.activation(stateT[hr, sco:sco + n], ps_e[hr, 0:n],
                                             AFT.Identity, bias=w["edge_bc"][hr, 0:1])
                for c0 in range(0, NCRYSP, 512):
                    ps_bc = psp.tile([128, 512], F32, tag="ps_e")
                    nc.tensor.matmul(ps_bc[:, :], w["ones1f"][:],
                                     w["iota1024f"][:, c0:c0 + 512],
                                     start=True, stop=True)
                    nc.vector.tensor_copy(iotaB[:, c0:c0 + 512], ps_bc[:, :])
                node_tables(0, sbp, psp)

            # ---- layers ----
            for l in range(L):
                with tc.tile_pool(name=f"sbe{l}", bufs=3) as sbp, \
                     tc.tile_pool(name=f"pse{l}", bufs=2, space="PSUM") as psp, \
                     tc.tile_pool(name=f"psb{l}", bufs=1, space="PSUM") as psb, \
                     tc.tile_pool(name=f"psg{l}", bufs=1, space="PSUM") as psg:
                    for blk in blocks:
                        b = blk["b"]
                        hr = slice(64, 128) if blk["half"] else slice(0, 64)
                        ba = min(128, ASH - 128 * b)
                        asrc_full = asrc_fulls[l]
                        gt = sbp.tile([128, 1, blk["nblk_e"]], BF16, tag="gt", bufs=2)
                        if blk["n_lo"] and not no_gather:
                            io = sum(bb["n_lo"] for bb in blocks[:b]) // 16
                            nc.gpsimd.dma_gather(
                                gt[:, :, 0:blk["n_lo"]], asrc_full[0:LOS, :],
                                idxsb[:, io:io + blk["n_lo"] // 16],
                                blk["n_lo"], blk["n_lo"], ND, transpose=True,
                                queue_num=(2 * b) % 4)
                        if blk["n_hi"] and not no_gather:
                            io = sum(bb["n_hi"] for bb in blocks[:b]) // 16
                            nc.gpsimd.dma_gather(
                                gt[:, :, blk["n_lo"]:], asrc_full[LOS:N, :],
                                idxsbh[:, io:io + blk["n_hi"] // 16],
                                blk["n_hi"], blk["n_hi"], ND, transpose=True,
                                queue_num=(2 * b + 1) % 4)
                        ps_agg = psg.tile([128, 64], F32, tag="agg")
                        nchunk = len(blk["chunks"])
                        e0 = blk["chunks"][0][1]
                        arsb = sbp.tile([1, blk["nblk_e"]], BF16, tag="arsb", bufs=2)
                        nc.sync.dma_start(arsb[0:1, :], arow_r[0:1, e0:e0 + blk["nblk_e"]])
                        for ci, (sco, eco, bco, n) in enumerate(blk["chunks"]):
                            tcol = eco // 128
                            ps_b = psb.tile([128, 512], F32, tag="pb")
                            nc.tensor.matmul(ps_b[:, 0:n], w["ones1b"][:],
                                             arsb[0:1, bco:bco + n], start=True, stop=True)
                            sst = sbp.tile([128, 512], BF16, tag="sst")
                            nc.vector.tensor_scalar(sst[:, 0:n], ps_b[:, 0:n],
                                                    w["iota_col"][:, 0:1], None, ALU.is_equal)
                            sse = sbp.tile([128, 512], BF16, tag="sse")
                            for j in range(n // 128):
                                nc.vector.tensor_scalar(sse[:, 128 * j:128 * j + 128],
                                                        w["iotaM"][:],
                                                        w["arow_w"][:, tcol + j:tcol + j + 1],
                                                        w["mask_w"][:, tcol + j:tcol + j + 1],
                                                        ALU.is_equal, ALU.mult)
                            ps_h = psp.tile([128, 512], F32, tag="ph")
                            nc.tensor.matmul(ps_h[:, 0:n], adst[0:ba, 128 * b:128 * b + 128],
                                             sst[0:ba, 0:n], start=True, stop=False)
                            nc.tensor.matmul(ps_h[:, 0:n], w["we_dup"][hr, l * HID:(l + 1) * HID],
                                             stateT[hr, sco:sco + n], start=False,
                                             stop=no_gather)
                            if not no_gather:
                                nc.tensor.matmul(ps_h[:, 0:n], w["i128b"][:],
                                                 gt[:, 0, bco:bco + n], start=False, stop=True)
                            ht = sbp.tile([128, 512], BF16, tag="ht")
                            nc.scalar.activation(ht[:, 0:n], ps_h[:, 0:n], act,
                                                 bias=w["eb1"][:, l:l + 1])
                            ps_dd = psp.tile([128, 512], F32, tag="pd")
                            nc.tensor.matmul(ps_dd[hr, 0:n], w["i64d"][hr, :],
                                             stateT[hr, sco:sco + n], start=True, stop=False)
                            nc.tensor.matmul(ps_dd[hr, 0:n], w["ew2_all"][:, l * ED:(l + 1) * ED],
                                             ht[:, 0:n], start=False, stop=True)
                            nc.scalar.activation(stateT[hr, sco:sco + n], ps_dd[hr, 0:n],
                                                 AFT.Identity, bias=w["eb2p"][hr, l:l + 1])
                            ps_t = psp.tile([128, 256], BF16, tag="pt", bufs=1)
                            for j in range(n // 128):
                                nc.tensor.transpose(
                                    ps_t[:, 64 * j:64 * j + 64],
                                    stateT[hr, sco + 128 * j:sco + 128 * j + 128],
                                    w["i64d"][hr, :])
                            nn = sbp.tile([128, 256], BF16, tag="nn")
                            nc.vector.tensor_copy(nn[:, 0:64 * (n // 128)],
                                                  ps_t[:, 0:64 * (n // 128)])
                            for j in range(n // 128):
                                nc.tensor.matmul(
                                    ps_agg[:],
                                    sse[:, 128 * j:128 * j + 128],
                                    nn[:, 64 * j:64 * j + 64],
                                    start=(ci == 0 and j == 0),
                                    stop=(ci == nchunk - 1 and j == n // 128 - 1))
                        agnb = sbp.tile([128, 64], BF16, tag="agnb")
                        nc.scalar.activation(agnb[:], ps_agg[:], AFT.Identity,
                                             scale=w["invcnt"][:, b:b + 1])
                        ps_at = psp.tile([128, 128], BF16, tag="pat", bufs=1)
                        nc.tensor.transpose(ps_at[0:64, :], agnb[:], w["i128b"][:])
                        nc.vector.tensor_copy(aggF[0:64, 128 * b:128 * b + 128],
                                              ps_at[0:64, :])
                # node MLP (512-atom chunks) + next-layer tables
                with tc.tile_pool(name=f"sbn{l}", bufs=3) as sbp, \
                     tc.tile_pool(name=f"psn{l}", bufs=2, space="PSUM") as psp:
                    for c0 in range(0, ASH, 512):
                        nw = min(512, ASH - c0)
                        ps_hn = psp.tile([128, 512], F32, tag="hn")
                        nc.tensor.matmul(ps_hn[:, 0:nw],
                                         w["nw1a_all"][:, l * HID:(l + 1) * HID],
                                         nodeTb[:, c0:c0 + nw],
                                         start=True, stop=False)
                        nc.tensor.matmul(ps_hn[:, 0:nw],
                                         w["nw1b_dup"][0:64, l * HID:(l + 1) * HID],
                                         aggF[0:64, c0:c0 + nw],
                                         start=False, stop=True)
                        hn = sbp.tile([128, 512], BF16, tag="hn_s")
                        nc.scalar.activation(hn[:, 0:nw], ps_hn[:, 0:nw], act,
                                             bias=w["nb1"][:, l:l + 1])
                        ps_nd = psp.tile([128, 512], F32, tag="ndl")
                        nc.tensor.matmul(ps_nd[:, 0:nw],
                                         w["nw2_all"][:, l * ND:(l + 1) * ND],
                                         hn[:, 0:nw], start=True, stop=False)
                        nc.tensor.matmul(ps_nd[:, 0:nw], w["i128f"][:],
                                         nodeT[:, c0:c0 + nw], start=False, stop=True)
                        nc.scalar.activation(nodeT[:, c0:c0 + nw], ps_nd[:, 0:nw],
                                             AFT.Identity, bias=w["nb2"][:, l:l + 1])
                        nc.vector.tensor_copy(nodeTb[:, c0:c0 + nw],
                                              nodeT[:, c0:c0 + nw])
                    if l < L - 1:
                        node_tables(l + 1, sbp, psp)

            # ---- pooling: one-hot over padded 1024-crystal axis, built on device.
            # Two sweeps of 4 crystal groups each (4 PSUM accumulator tiles/sweep).
            for sweep in range(2):
                g0 = sweep * (NGP // 2)
                with tc.tile_pool(name=f"sbp{sweep}", bufs=3) as sbp, \
                     tc.tile_pool(name=f"psp{sweep}", bufs=1, space="PSUM") as psp, \
                     tc.tile_pool(name=f"psq{sweep}", bufs=2, space="PSUM") as psq:
                    pools = [psp.tile([128, 128], F32, tag=f"pool{g}", name=f"pool{sweep}_{g}")
                             for g in range(NGP // 2)]
                    for t in range(NBLK):
                        na = min(128, ASH - 128 * t)
                        poh = sbp.tile([128, 512], BF16, tag="poh", bufs=2)
                        nc.vector.tensor_scalar(poh[:, :], iotaB[:, 512 * sweep:512 * sweep + 512],
                                                w["cbcol"][:, t:t + 1], None, ALU.is_equal)
                        ps_tr = psq.tile([128, 128], F32, tag="ptr")
                        nc.tensor.transpose(ps_tr[0:na, :], nodeT[:, 128 * t:128 * t + na],
                                            w["i128f"][:])
                        nnat = sbp.tile([128, 128], BF16, tag="nnat")
                        nc.vector.tensor_copy(nnat[0:na, :], ps_tr[0:na, :])
                        for g in range(NGP // 2):
                            nc.tensor.matmul(pools[g][:, :],
                                             poh[0:na, 128 * g:128 * g + 128],
                                             nnat[0:na, :],
                                             start=(t == 0), stop=(t == NBLK - 1))
                    for g in range(NGP // 2):
                        pev = sbp.tile([128, 128], F32, tag="pev")
                        nc.vector.tensor_copy(pev[:, :], pools[g][:, :])
                        nc.sync.dma_start(pool_in[128 * (g0 + g):128 * (g0 + g) + 128, :],
                                          pev[:, :])
            if not no_coll:
                nc.gpsimd.collective_compute(
                    "AllReduce", mybir.AluOpType.add,
                    replica_groups=[list(range(NCORES))],
                    ins=[pool_in[:].opt()], outs=[pool_out[:].opt()],
                )

            # ---- readout (replicated): mean-scale, transpose, 2-layer head ----
            with tc.tile_pool(name="sbr", bufs=2) as sbp, \
                 tc.tile_pool(name="psr", bufs=2, space="PSUM") as psp:
                for g in range(NGP):
                    gc = min(128, NCRYS - 128 * g)
                    if gc <= 0:
                        break
                    pob = sbp.tile([128, 128], F32, tag="pob")
                    nc.sync.dma_start(pob[:], pool_out[128 * g:128 * g + 128, :])
                    pmb = sbp.tile([128, 128], BF16, tag="pmb")
                    nc.scalar.activation(pmb[:, :], pob[:, :], AFT.Identity,
                                         scale=w["invccnt8"][:, g:g + 1])
                    ps_t2 = psp.tile([128, 128], BF16, tag="pt2", bufs=1)
                    nc.tensor.transpose(ps_t2[:, :], pmb[:, :], w["i128b"][:])
                    meanT = sbp.tile([128, 128], BF16, tag="meanT")
                    nc.vector.tensor_copy(meanT[:, :], ps_t2[:, :])
                    ps_hr = psp.tile([128, 128], F32, tag="phr")
                    nc.tensor.matmul(ps_hr[:, 0:gc], w["readW"][:], meanT[:, 0:gc],
                                     start=True, stop=True)
                    hrT = sbp.tile([128, 128], BF16, tag="hrT")
                    nc.scalar.activation(hrT[:, 0:gc], ps_hr[:, 0:gc], act,
                                         bias=w["readb"][:])
                    ps_y = psp.tile([128, 128], F32, tag="py")
                    nc.tensor.matmul(ps_y[0:1, 0:gc], w["outW"][:], hrT[:, 0:gc],
                                     start=True, stop=True)
                    ysb = sbp.tile([1, 128], F32, tag="ysb")
                    nc.scalar.activation(ysb[0:1, 0:gc], ps_y[0:1, 0:gc], AFT.Copy,
                                         bias=meta["out_b"])
                    nc.sync.dma_start(y[0:1, 128 * g:128 * g + gc], ysb[0:1, 0:gc])

    nc.compile()
    return nc


def run_cores(meta, in_maps, act=AFT.Silu, sim=False):
    nc = _build(meta, act=act)
    if sim:
        from concourse.bass_interp import MultiCoreSim
        s = MultiCoreSim(nc, NCORES, trace=False)
        for k in range(NCORES):
            for nm, arr in in_maps[k].items():
                s.cores[k].tensor(nm)[:] = arr
        s.simulate(check_with_hw=False)
        return [{"y": np.array(s.cores[k].tensor("y"))} for k in range(NCORES)], None
    from concourse import bass_utils
    res = bass_utils.run_bass_kernel_spmd(nc, in_maps, core_ids=list(range(NCORES)))
    return res.results, res


def kernel(**inputs):
    cfg = dict(FULL_CFG)
    n, m = np.asarray(inputs["nbr_fea_idx"]).shape
    cfg["N"], cfg["M"] = int(n), int(m)
    cfg["AFD"] = int(np.asarray(inputs["atom_fea"]).shape[1])
    cfg["EFD"] = int(np.asarray(inputs["nbr_fea"]).shape[2])
    cfg["NCRYS"] = int(inputs["num_crystals"])
    cfg["L"] = int(np.asarray(inputs["eW1"]).shape[0])
    meta, in_maps = _prep(inputs, cfg)
    results, _ = run_cores(meta, in_maps)
    return np.asarray(results[0]["y"], np.float32).reshape(cfg["NCRYS"], 1)
